# revision 2
# baseline (speedup 1.0000x reference)
"""2-layer IndRNN (diagonal recurrence) + linear head on 8 trn2 NeuronCores.

v2 strategy (data-parallel over batch, 32 rows/core, ONE chunk of BC=32):
  - Feature-major layout [h_inner=partition, (o, t, b)=free].
  - GEMM-0: f32r matmul per 16-t block, per m-tile; PSUM->SBUF copy on Act
    fuses bias b0 and fp16 convert -> pre0 ring.
  - Recurrence keeps fp16 pre-activation state z_t in place in the pre ring:
    tm = (z_{t-1} max 0) * u  (DVE scalar_tensor_tensor, 1 elem/cyc),
    z_t = tm + pre_t          (DVE tensor_tensor add, fp16 2x mode).
  - h0 = relu(z0) -> fp8e4 ring, per-4-m-tile ops on GPSIMD (idle engine).
  - GEMM-1 in fp8e4 DoubleRow perf mode: 8 k-pair matmuls per m-tile per
    block (2 k-tiles per instruction); W1 pre-scaled x64 on host; the Act
    PSUM->SBUF copy applies scale=1/64 + bias b1 + fp16 convert.
  - Recurrence 1 in place in the pre1 ring; head = relu(z1[T-1]) -> f32r,
    16 accumulated [128,1]x[128,BC] matmuls + lin_b bias.
Host side only reorders/shards numpy inputs; all FLOPs run on device.
"""

import numpy as np

B, T, I, H = 256, 100, 128, 2048
NCORES = 8
BL = B // NCORES            # batch rows per core
BC = BL                     # one chunk
NO = H // 128               # 16 h-tiles
NKP = NO // 2               # 8 fp8 k-pairs
TBLKS = [(0, 16), (16, 16), (32, 16), (48, 16), (64, 16), (80, 16), (96, 4)]
S1 = 64.0                   # fp8 weight pre-scale for W1
FP8 = True                  # flip to False for bf16 GEMM-1 fallback
CONV_ON_GP = True           # h0 relu+fp8 convert on GPSIMD vs Act

_CACHE = {}


def _build():
    import concourse.tile as tile
    from concourse import bacc, mybir

    f32 = mybir.dt.float32
    f16 = mybir.dt.float16
    bf16 = mybir.dt.bfloat16
    f32r = mybir.dt.float32r
    f8 = mybir.dt.float8e4
    g1dt = f8 if FP8 else bf16
    RELU = mybir.ActivationFunctionType.Relu
    IDENT = mybir.ActivationFunctionType.Identity
    MAX = mybir.AluOpType.max
    MULT = mybir.AluOpType.mult
    DR = mybir.MatmulPerfMode.DoubleRow

    nc = bacc.Bacc(None, target_bir_lowering=False)

    xT_d = nc.dram_tensor("xT", [128, T, BC], f32r, kind="ExternalInput")
    w0T_d = nc.dram_tensor("w0T", [128, NO, 128], f32r, kind="ExternalInput")
    w1T_d = nc.dram_tensor("w1T", [128, NO, NO, 128], g1dt, kind="ExternalInput")
    u0f_d = nc.dram_tensor("u0f", [128, NO, BC], f16, kind="ExternalInput")
    u1f_d = nc.dram_tensor("u1f", [128, NO, BC], f16, kind="ExternalInput")
    b0_d = nc.dram_tensor("b0t", [128, NO], f32, kind="ExternalInput")
    b1_d = nc.dram_tensor("b1t", [128, NO], f32, kind="ExternalInput")
    lw_d = nc.dram_tensor("lwt", [128, NO], f32r, kind="ExternalInput")
    lb_d = nc.dram_tensor("lbt", [1, 1], f32, kind="ExternalInput")
    out_d = nc.dram_tensor("out", [1, BL], f32, kind="ExternalOutput")

    with tile.TileContext(nc) as tc:
        with (
            tc.tile_pool(name="const", bufs=1) as const,
            tc.tile_pool(name="p0", bufs=3) as p0p,
            tc.tile_pool(name="p0t", bufs=1) as p0tp,
            tc.tile_pool(name="h0", bufs=3) as h0p,
            tc.tile_pool(name="h0t", bufs=1) as h0tp,
            tc.tile_pool(name="p1", bufs=2) as p1p,
            tc.tile_pool(name="p1t", bufs=1) as p1tp,
            tc.tile_pool(name="tmp", bufs=4) as tmp,
            tc.tile_pool(name="ps0", bufs=3, space="PSUM") as ps0,
            tc.tile_pool(name="ps1", bufs=4, space="PSUM") as ps1,
        ):
            xs = const.tile([128, T, BC], f32r, tag="xs")
            w0T = const.tile([128, NO, 128], f32r, tag="w0T")
            w1T = const.tile([128, NO, NO, 128], g1dt, tag="w1T")
            u0f = const.tile([128, NO, BC], f16, tag="u0f")
            u1f = const.tile([128, NO, BC], f16, tag="u1f")
            b0t = const.tile([128, NO], f32, tag="b0t")
            b1t = const.tile([128, NO], f32, tag="b1t")
            lwt = const.tile([128, NO], f32r, tag="lwt")
            lbt = const.tile([1, 1], f32, tag="lbt")
            outs = const.tile([1, BL], f32, tag="outs")

            nc.sync.dma_start(out=xs[:], in_=xT_d[:])
            nc.sync.dma_start(out=w0T[:], in_=w0T_d[:])
            for kb in range(NO):
                nc.sync.dma_start(out=w1T[:, kb], in_=w1T_d[:, kb])
            nc.sync.dma_start(out=u0f[:], in_=u0f_d[:])
            nc.sync.dma_start(out=u1f[:], in_=u1f_d[:])
            nc.sync.dma_start(out=b0t[:], in_=b0_d[:])
            nc.sync.dma_start(out=b1t[:], in_=b1_d[:])
            nc.sync.dma_start(out=lwt[:], in_=lw_d[:])
            nc.sync.dma_start(out=lbt[:], in_=lb_d[:])

            p0blks = []
            h0blks = []
            p1blks = []

            def emit_g0(nb):
                t0, TB = TBLKS[nb]
                pool = p0p if TB == 16 else p0tp
                pb = pool.tile([128, NO, TB, BC], f16,
                               tag="p0" if TB == 16 else "p0tl")
                p0blks.append(pb)
                for m in range(NO):
                    ps = ps0.tile([128, 16, BC], f32, tag="ps0")
                    nc.tensor.matmul(
                        ps[:, :TB], w0T[:, m], xs[:, t0:t0 + TB],
                        start=True, stop=True,
                    )
                    nc.scalar.activation(
                        pb[:, m], ps[:, :TB], IDENT,
                        bias=b0t[:, m:m + 1], scale=1.0,
                    )

            def emit_rec0(nb):
                t0, TB = TBLKS[nb]
                pb = p0blks[nb]
                for trel in range(TB):
                    t = t0 + trel
                    if t == 0:
                        continue
                    cur = pb[:, :, trel]
                    pbb, pt = ((t - 1) >> 4), ((t - 1) & 15)
                    prev = p0blks[pbb][:, :, pt]
                    tm = tmp.tile([128, NO, BC], f16, tag="tm0")
                    nc.vector.scalar_tensor_tensor(
                        tm[:], prev, 0.0, u0f[:], MAX, MULT,
                    )
                    nc.vector.tensor_add(cur, tm[:], cur)

            def emit_conv(nb):
                t0, TB = TBLKS[nb]
                pool = h0p if TB == 16 else h0tp
                hb = pool.tile([128, NO, TB, BC], g1dt,
                               tag="h0" if TB == 16 else "h0tl")
                h0blks.append(hb)
                eng = nc.gpsimd if CONV_ON_GP else nc.scalar
                for mg in range(4):
                    sl = slice(mg * 4, (mg + 1) * 4)
                    if CONV_ON_GP:
                        nc.gpsimd.tensor_scalar_max(
                            hb[:, sl], p0blks[nb][:, sl], 0.0,
                        )
                    else:
                        nc.scalar.activation(hb[:, sl], p0blks[nb][:, sl], RELU)

            def emit_g1(nb):
                t0, TB = TBLKS[nb]
                pool = p1p if TB == 16 else p1tp
                rb = pool.tile([128, NO, TB, BC], f16,
                               tag="p1" if TB == 16 else "p1tl")
                p1blks.append(rb)
                for m in range(NO):
                    ps = ps1.tile([128, 16, BC], f32, tag="ps1")
                    if FP8:
                        for kp in range(NKP):
                            nc.tensor.matmul(
                                ps[:, :TB],
                                w1T[:, 2 * kp:2 * kp + 2, m],
                                h0blks[nb][:, 2 * kp:2 * kp + 2],
                                start=(kp == 0), stop=(kp == NKP - 1),
                                perf_mode=DR,
                            )
                    else:
                        for k in range(NO):
                            nc.tensor.matmul(
                                ps[:, :TB], w1T[:, k, m], h0blks[nb][:, k],
                                start=(k == 0), stop=(k == NO - 1),
                            )
                    nc.scalar.activation(
                        rb[:, m], ps[:, :TB], IDENT,
                        bias=b1t[:, m:m + 1], scale=(1.0 / S1) if FP8 else 1.0,
                    )

            def emit_rec1(nb):
                t0, TB = TBLKS[nb]
                rb = p1blks[nb]
                for trel in range(TB):
                    t = t0 + trel
                    if t == 0:
                        continue
                    cur = rb[:, :, trel]
                    pbb, pt = ((t - 1) >> 4), ((t - 1) & 15)
                    prev = p1blks[pbb][:, :, pt]
                    tm = tmp.tile([128, NO, BC], f16, tag="tm1")
                    nc.vector.scalar_tensor_tensor(
                        tm[:], prev, 0.0, u1f[:], MAX, MULT,
                    )
                    nc.vector.tensor_add(cur, tm[:], cur)

            nblk = len(TBLKS)
            for nb in range(nblk):
                emit_g0(nb)
                emit_rec0(nb)
                emit_conv(nb)
                if nb >= 1:
                    emit_g1(nb - 1)
                    emit_rec1(nb - 1)
            emit_g1(nblk - 1)
            emit_rec1(nblk - 1)

            # head: out[b] = lin_w . relu(z1_{T-1}) + lin_b
            lt0, lTB = TBLKS[-1]
            h1h = tmp.tile([128, NO, BC], f32r, tag="h1h")
            nc.scalar.activation(h1h[:], p1blks[-1][:, :, lTB - 1], RELU)
            ph = ps0.tile([128, 16, BC], f32, tag="ps0")
            for o in range(NO):
                nc.tensor.matmul(
                    ph[0:1, 0], lwt[:, o:o + 1], h1h[:, o],
                    start=(o == 0), stop=(o == NO - 1),
                )
            nc.scalar.activation(
                outs[0:1, :], ph[0:1, 0], IDENT,
                bias=lbt[0:1, 0:1], scale=1.0,
            )
            nc.sync.dma_start(out=out_d[:], in_=outs[:])

    nc.compile()
    return nc


def _get_nc():
    if "nc" not in _CACHE:
        _CACHE["nc"] = _build()
    return _CACHE["nc"]


def _trunc22(a):
    return (np.ascontiguousarray(a).view(np.int32) & np.int32(~0x3FF)).view(np.float32)


def _prep_shared(W0, b0, u0, W1, b1, u1, lin_w, lin_b):
    import ml_dtypes

    w0T = _trunc22(np.ascontiguousarray(W0.T)).reshape(128, NO, 128)
    w1g = W1 * S1 if FP8 else W1
    w1dt = ml_dtypes.float8_e4m3 if FP8 else ml_dtypes.bfloat16
    w1T = np.ascontiguousarray(
        w1g.reshape(NO, 128, NO, 128).transpose(3, 2, 0, 1)
    ).astype(w1dt)
    u0f = np.ascontiguousarray(
        np.broadcast_to(u0.reshape(NO, 128).T[:, :, None], (128, NO, BC))
    ).astype(np.float16)
    u1f = np.ascontiguousarray(
        np.broadcast_to(u1.reshape(NO, 128).T[:, :, None], (128, NO, BC))
    ).astype(np.float16)
    b0t = np.ascontiguousarray(b0.reshape(NO, 128).T)
    b1t = np.ascontiguousarray(b1.reshape(NO, 128).T)
    lwt = _trunc22(np.ascontiguousarray(lin_w.reshape(NO, 128).T))
    lbt = np.ascontiguousarray(lin_b.reshape(1, 1))
    return dict(w0T=w0T, w1T=w1T, u0f=u0f, u1f=u1f,
                b0t=b0t, b1t=b1t, lwt=lwt, lbt=lbt)


def make_in_maps(x, W0, b0, u0, W1, b1, u1, lin_w, lin_b):
    shared = _prep_shared(
        np.asarray(W0, np.float32), np.asarray(b0, np.float32),
        np.asarray(u0, np.float32), np.asarray(W1, np.float32),
        np.asarray(b1, np.float32), np.asarray(u1, np.float32),
        np.asarray(lin_w, np.float32), np.asarray(lin_b, np.float32),
    )
    x = np.asarray(x, np.float32)
    in_maps = []
    for core in range(NCORES):
        xc = x[core * BL:(core + 1) * BL]            # (BL, T, I)
        xT = _trunc22(np.ascontiguousarray(xc.transpose(2, 1, 0)))
        in_maps.append({"xT": xT, **shared})
    return in_maps


def kernel(x, W0, b0, u0, W1, b1, u1, lin_w, lin_b):
    from concourse.bass_utils import run_bass_kernel_spmd

    nc = _get_nc()
    in_maps = make_in_maps(x, W0, b0, u0, W1, b1, u1, lin_w, lin_b)
    try:
        res = run_bass_kernel_spmd(nc, in_maps, list(range(NCORES)))
    except Exception:
        res = run_bass_kernel_spmd(nc, in_maps, list(range(NCORES)))
    return np.concatenate([r["out"][0] for r in res.results])


# revision 3
# speedup vs baseline: 1.8684x; 1.8684x over previous
"""2-layer IndRNN (diagonal recurrence) + linear head on 8 trn2 NeuronCores.

v2 strategy (data-parallel over batch, 32 rows/core, ONE chunk of BC=32):
  - Feature-major layout [h_inner=partition, (o, t, b)=free].
  - GEMM-0: f32r matmul per 16-t block, per m-tile; PSUM->SBUF copy on Act
    fuses bias b0 and fp16 convert -> pre0 ring.
  - Recurrence keeps fp16 pre-activation state z_t in place in the pre ring:
    tm = (z_{t-1} max 0) * u  (DVE scalar_tensor_tensor, 1 elem/cyc),
    z_t = tm + pre_t          (DVE tensor_tensor add, fp16 2x mode).
  - h0 = relu(z0) -> fp8e4 ring, per-4-m-tile ops on GPSIMD (idle engine).
  - GEMM-1 in fp8e4 DoubleRow perf mode: 8 k-pair matmuls per m-tile per
    block (2 k-tiles per instruction); W1 pre-scaled x64 on host; the Act
    PSUM->SBUF copy applies scale=1/64 + bias b1 + fp16 convert.
  - Recurrence 1 in place in the pre1 ring; head = relu(z1[T-1]) -> f32r,
    16 accumulated [128,1]x[128,BC] matmuls + lin_b bias.
Host side only reorders/shards numpy inputs; all FLOPs run on device.
"""

import numpy as np

B, T, I, H = 256, 100, 128, 2048
NCORES = 8
BL = B // NCORES            # batch rows per core
BC = BL                     # one chunk
NO = H // 128               # 16 h-tiles
NKP = NO // 2               # 8 fp8 k-pairs
TBLKS = [(0, 16), (16, 16), (32, 16), (48, 16), (64, 16), (80, 16), (96, 4)]
S1 = 64.0                   # fp8 weight pre-scale for W1
FP8 = True                  # flip to False for bf16 GEMM-1 fallback
CONV_ON_GP = False          # h0 relu+fp8 convert on GPSIMD vs Act

_CACHE = {}


def _build():
    import concourse.tile as tile
    from concourse import bacc, mybir

    f32 = mybir.dt.float32
    f16 = mybir.dt.float16
    bf16 = mybir.dt.bfloat16
    f32r = mybir.dt.float32r
    f8 = mybir.dt.float8e4
    g1dt = f8 if FP8 else bf16
    RELU = mybir.ActivationFunctionType.Relu
    IDENT = mybir.ActivationFunctionType.Identity
    MAX = mybir.AluOpType.max
    MULT = mybir.AluOpType.mult
    DR = mybir.MatmulPerfMode.DoubleRow

    nc = bacc.Bacc(None, target_bir_lowering=False)

    xT_d = nc.dram_tensor("xT", [128, T, BC], f32r, kind="ExternalInput")
    w0T_d = nc.dram_tensor("w0T", [128, NO, 128], f32r, kind="ExternalInput")
    w1T_d = nc.dram_tensor("w1T", [128, NO, NO, 128], g1dt, kind="ExternalInput")
    u0f_d = nc.dram_tensor("u0f", [128, NO, BC], f16, kind="ExternalInput")
    u1f_d = nc.dram_tensor("u1f", [128, NO, BC], f16, kind="ExternalInput")
    b0_d = nc.dram_tensor("b0t", [128, NO], f32, kind="ExternalInput")
    b1_d = nc.dram_tensor("b1t", [128, NO], f32, kind="ExternalInput")
    lw_d = nc.dram_tensor("lwt", [128, NO], f32r, kind="ExternalInput")
    lb_d = nc.dram_tensor("lbt", [1, 1], f32, kind="ExternalInput")
    out_d = nc.dram_tensor("out", [1, BL], f32, kind="ExternalOutput")

    with tile.TileContext(nc) as tc:
        with (
            tc.tile_pool(name="const", bufs=1) as const,
            tc.tile_pool(name="p0", bufs=3) as p0p,
            tc.tile_pool(name="p0t", bufs=1) as p0tp,
            tc.tile_pool(name="h0", bufs=3) as h0p,
            tc.tile_pool(name="h0t", bufs=1) as h0tp,
            tc.tile_pool(name="p1", bufs=2) as p1p,
            tc.tile_pool(name="p1t", bufs=1) as p1tp,
            tc.tile_pool(name="tmp", bufs=4) as tmp,
            tc.tile_pool(name="ps0", bufs=3, space="PSUM") as ps0,
            tc.tile_pool(name="ps1", bufs=4, space="PSUM") as ps1,
        ):
            xs = const.tile([128, T, BC], f32r, tag="xs")
            w0T = const.tile([128, NO, 128], f32r, tag="w0T")
            w1T = const.tile([128, NO, NO, 128], g1dt, tag="w1T")
            u0f = const.tile([128, NO, BC], f16, tag="u0f")
            u1f = const.tile([128, NO, BC], f16, tag="u1f")
            b0t = const.tile([128, NO], f32, tag="b0t")
            b1t = const.tile([128, NO], f32, tag="b1t")
            lwt = const.tile([128, NO], f32r, tag="lwt")
            lbt = const.tile([1, 1], f32, tag="lbt")
            outs = const.tile([1, BL], f32, tag="outs")

            nc.sync.dma_start(out=xs[:], in_=xT_d[:])
            nc.sync.dma_start(out=w0T[:], in_=w0T_d[:])
            for kb in range(NO):
                nc.sync.dma_start(out=w1T[:, kb], in_=w1T_d[:, kb])
            nc.sync.dma_start(out=u0f[:], in_=u0f_d[:])
            nc.sync.dma_start(out=u1f[:], in_=u1f_d[:])
            nc.sync.dma_start(out=b0t[:], in_=b0_d[:])
            nc.sync.dma_start(out=b1t[:], in_=b1_d[:])
            nc.sync.dma_start(out=lwt[:], in_=lw_d[:])
            nc.sync.dma_start(out=lbt[:], in_=lb_d[:])

            p0blks = []
            h0blks = []
            p1blks = []

            def emit_g0(nb):
                t0, TB = TBLKS[nb]
                pool = p0p if TB == 16 else p0tp
                pb = pool.tile([128, NO, TB, BC], f16,
                               tag="p0" if TB == 16 else "p0tl")
                p0blks.append(pb)
                for m in range(NO):
                    ps = ps0.tile([128, 16, BC], f32, tag="ps0")
                    nc.tensor.matmul(
                        ps[:, :TB], w0T[:, m], xs[:, t0:t0 + TB],
                        start=True, stop=True,
                    )
                    nc.scalar.activation(
                        pb[:, m], ps[:, :TB], IDENT,
                        bias=b0t[:, m:m + 1], scale=1.0,
                    )

            def emit_rec0(nb):
                t0, TB = TBLKS[nb]
                pb = p0blks[nb]
                for trel in range(TB):
                    t = t0 + trel
                    if t == 0:
                        continue
                    cur = pb[:, :, trel]
                    pbb, pt = ((t - 1) >> 4), ((t - 1) & 15)
                    prev = p0blks[pbb][:, :, pt]
                    tm = tmp.tile([128, NO, BC], f16, tag="tm0")
                    nc.vector.scalar_tensor_tensor(
                        tm[:], prev, 0.0, u0f[:], MAX, MULT,
                    )
                    nc.vector.tensor_add(cur, tm[:], cur)

            def emit_conv(nb):
                t0, TB = TBLKS[nb]
                pool = h0p if TB == 16 else h0tp
                hb = pool.tile([128, NO, TB, BC], g1dt,
                               tag="h0" if TB == 16 else "h0tl")
                h0blks.append(hb)
                eng = nc.gpsimd if CONV_ON_GP else nc.scalar
                for mg in range(4):
                    sl = slice(mg * 4, (mg + 1) * 4)
                    if CONV_ON_GP:
                        nc.gpsimd.tensor_scalar_max(
                            hb[:, sl], p0blks[nb][:, sl], 0.0,
                        )
                    else:
                        nc.scalar.activation(hb[:, sl], p0blks[nb][:, sl], RELU)

            def emit_g1(nb):
                t0, TB = TBLKS[nb]
                pool = p1p if TB == 16 else p1tp
                rb = pool.tile([128, NO, TB, BC], f16,
                               tag="p1" if TB == 16 else "p1tl")
                p1blks.append(rb)
                for m in range(NO):
                    ps = ps1.tile([128, 16, BC], f32, tag="ps1")
                    if FP8:
                        for kp in range(NKP):
                            nc.tensor.matmul(
                                ps[:, :TB],
                                w1T[:, 2 * kp:2 * kp + 2, m],
                                h0blks[nb][:, 2 * kp:2 * kp + 2],
                                start=(kp == 0), stop=(kp == NKP - 1),
                                perf_mode=DR,
                            )
                    else:
                        for k in range(NO):
                            nc.tensor.matmul(
                                ps[:, :TB], w1T[:, k, m], h0blks[nb][:, k],
                                start=(k == 0), stop=(k == NO - 1),
                            )
                    nc.scalar.activation(
                        rb[:, m], ps[:, :TB], IDENT,
                        bias=b1t[:, m:m + 1], scale=(1.0 / S1) if FP8 else 1.0,
                    )

            def emit_rec1(nb):
                t0, TB = TBLKS[nb]
                rb = p1blks[nb]
                for trel in range(TB):
                    t = t0 + trel
                    if t == 0:
                        continue
                    cur = rb[:, :, trel]
                    pbb, pt = ((t - 1) >> 4), ((t - 1) & 15)
                    prev = p1blks[pbb][:, :, pt]
                    tm = tmp.tile([128, NO, BC], f16, tag="tm1")
                    nc.vector.scalar_tensor_tensor(
                        tm[:], prev, 0.0, u1f[:], MAX, MULT,
                    )
                    nc.vector.tensor_add(cur, tm[:], cur)

            nblk = len(TBLKS)
            for nb in range(nblk):
                emit_g0(nb)
                emit_rec0(nb)
                emit_conv(nb)
                if nb >= 1:
                    emit_g1(nb - 1)
                    emit_rec1(nb - 1)
            emit_g1(nblk - 1)
            emit_rec1(nblk - 1)

            # head: out[b] = lin_w . relu(z1_{T-1}) + lin_b
            lt0, lTB = TBLKS[-1]
            h1h = tmp.tile([128, NO, BC], f32r, tag="h1h")
            nc.scalar.activation(h1h[:], p1blks[-1][:, :, lTB - 1], RELU)
            ph = ps0.tile([128, 16, BC], f32, tag="ps0")
            for o in range(NO):
                nc.tensor.matmul(
                    ph[0:1, 0], lwt[:, o:o + 1], h1h[:, o],
                    start=(o == 0), stop=(o == NO - 1),
                )
            nc.scalar.activation(
                outs[0:1, :], ph[0:1, 0], IDENT,
                bias=lbt[0:1, 0:1], scale=1.0,
            )
            nc.sync.dma_start(out=out_d[:], in_=outs[:])

    nc.compile()
    return nc


def _get_nc():
    if "nc" not in _CACHE:
        _CACHE["nc"] = _build()
    return _CACHE["nc"]


def _trunc22(a):
    return (np.ascontiguousarray(a).view(np.int32) & np.int32(~0x3FF)).view(np.float32)


def _prep_shared(W0, b0, u0, W1, b1, u1, lin_w, lin_b):
    import ml_dtypes

    w0T = _trunc22(np.ascontiguousarray(W0.T)).reshape(128, NO, 128)
    w1g = W1 * S1 if FP8 else W1
    w1dt = ml_dtypes.float8_e4m3 if FP8 else ml_dtypes.bfloat16
    w1T = np.ascontiguousarray(
        w1g.reshape(NO, 128, NO, 128).transpose(3, 2, 0, 1)
    ).astype(w1dt)
    u0f = np.ascontiguousarray(
        np.broadcast_to(u0.reshape(NO, 128).T[:, :, None], (128, NO, BC))
    ).astype(np.float16)
    u1f = np.ascontiguousarray(
        np.broadcast_to(u1.reshape(NO, 128).T[:, :, None], (128, NO, BC))
    ).astype(np.float16)
    b0t = np.ascontiguousarray(b0.reshape(NO, 128).T)
    b1t = np.ascontiguousarray(b1.reshape(NO, 128).T)
    lwt = _trunc22(np.ascontiguousarray(lin_w.reshape(NO, 128).T))
    lbt = np.ascontiguousarray(lin_b.reshape(1, 1))
    return dict(w0T=w0T, w1T=w1T, u0f=u0f, u1f=u1f,
                b0t=b0t, b1t=b1t, lwt=lwt, lbt=lbt)


def make_in_maps(x, W0, b0, u0, W1, b1, u1, lin_w, lin_b):
    shared = _prep_shared(
        np.asarray(W0, np.float32), np.asarray(b0, np.float32),
        np.asarray(u0, np.float32), np.asarray(W1, np.float32),
        np.asarray(b1, np.float32), np.asarray(u1, np.float32),
        np.asarray(lin_w, np.float32), np.asarray(lin_b, np.float32),
    )
    x = np.asarray(x, np.float32)
    in_maps = []
    for core in range(NCORES):
        xc = x[core * BL:(core + 1) * BL]            # (BL, T, I)
        xT = _trunc22(np.ascontiguousarray(xc.transpose(2, 1, 0)))
        in_maps.append({"xT": xT, **shared})
    return in_maps


def kernel(x, W0, b0, u0, W1, b1, u1, lin_w, lin_b):
    from concourse.bass_utils import run_bass_kernel_spmd

    nc = _get_nc()
    in_maps = make_in_maps(x, W0, b0, u0, W1, b1, u1, lin_w, lin_b)
    try:
        res = run_bass_kernel_spmd(nc, in_maps, list(range(NCORES)))
    except Exception:
        res = run_bass_kernel_spmd(nc, in_maps, list(range(NCORES)))
    return np.concatenate([r["out"][0] for r in res.results])
